# revision 25
# baseline (speedup 1.0000x reference)
"""Trainium2 Bass kernel for batched tanh-RNN (B=5000, T=8, V=5264, H=200).

  xh   = X @ W_ih.T + b_ih + b_hh          # [B, T, H]  (bulk of FLOPs)
  h_t  = tanh(xh[:, t] + h_{t-1} @ W_hh.T) # 8 steps
  out  = h_T @ W_fc.T + b_fc               # [B, V]

Strategy: data-parallel over batch across 8 NeuronCores (625 rows each),
weights replicated.  On each core everything is computed in "transposed"
layout (hidden dim on partitions, batch on the free dim) so the recurrence
needs no on-chip transposes:

  phase 1: xh.T[h, t*625+b] accumulated in PSUM over 42 v-tiles of 128;
           stationary = W_ih.T tiles, moving = X.T slabs streamed from HBM.
           X is re-laid-out on the host to [128, 42, 5000] (v-major) so the
           DMA is 2KB-contiguous per partition.
  phase 2: h.T = tanh(W_hh.T.T @ h.T + xh_t.T); the xh_t term is added into
           the same PSUM accumulation group via an identity-stationary
           matmul, then one ACT Tanh PSUM->SBUF per tile.
  phase 3: out[b, v] = h.T-as-stationary @ W_fc.T tiles (+ b_fc via a
           ones-stationary K=1 matmul), natural-layout DMA store.

All matmul operands are float32r (FP22 single-pass, 1 cycle/row for moving
free dim >= 256) — ~1e-4 relative precision, full PE speed.
"""

import numpy as np

import concourse.bass as bass
import concourse.mybir as mybir
from concourse import bacc
from concourse.bass_utils import run_bass_kernel_spmd
from concourse.tile import TileContext

NCORE = 8
B, T, V, H = 5000, 8, 5264, 200
Bc = B // NCORE            # 625 batch rows per core
Bp = 640                   # padded to keep all fp32r APs 8-byte aligned
BT = Bp * T                # 5120 (t-major columns: col = t*Bp + b)
VP = 5376                  # V padded to 42*128
KT = VP // 128             # 42 contraction tiles
SUB = 14                   # v-tiles per streamed X slab
NSUB = KT // SUB           # 3 slabs per bt-chunk
HA, HB = 128, H - 128      # hidden split across partition tiles (128 + 72)

# Mixed-precision phase 1: timesteps 0..T8-1 stream X in fp8-e4m3 and run
# DoubleRow matmuls (2 fp8 K-rows/cycle, ~1.7x bf16); errors injected that
# early are attenuated to noise by the 2+ remaining rounds of tanh
# saturation (measured: rel_absmax 6.5e-3 vs 4.1e-3 all-bf16, gate 2e-2).
# The last T16 timesteps stay bf16.
# Chunk widths keep DMA descriptors >= 512 B (below that SDMA does RMW at
# half rate): fp8 1 B/col -> 512-col chunks; bf16 2 B/col -> 512/256 ok.
T8 = 6                     # timesteps in fp8
T16 = T - T8               # timesteps in bf16
C8 = Bp * T8               # fp8 column region [0, 3840)
CHUNKS8 = [512] * 7 + [256]           # 3840 cols
CHUNKS16 = [512, 512, 256]            # 1280 cols
NCH8 = len(CHUNKS8)
HP8 = 208                  # W_ih fp8 h-stride padded to 16B multiple

F32 = mybir.dt.float32
F32R = mybir.dt.float32r
BF16 = mybir.dt.bfloat16
FP8 = mybir.dt.float8e4
AF = mybir.ActivationFunctionType
DR = mybir.MatmulPerfMode.DoubleRow

# recurrence b-chunks (even, >=256 so float32r runs 1 cycle/row)
REC_CHUNKS = [(0, 320), (320, 320)]
# FC output v-chunks (even offsets/widths, all >=256, <=512)
FC_CHUNKS = [(i * 480, 480) for i in range(10)] + [(4800, 464)]
# FC batch tiles over padded 640 (stationary free dim = 128; the last tile
# computes 15 pad rows that are simply not stored)
FC_BTILES = [(0, 128, 128), (128, 128, 128), (256, 128, 128),
             (384, 128, 128), (512, 128, 113)]

_CACHE = {}
LAST_RESULT = None  # BassKernelResults of the most recent run (for test.py)


def _build(reps=1, bench_internal=False, phases=3, sub=SUB, xbufs=3, fc_mode='full',
           yout_internal=False):
    # Bacc (not raw Bass): its finalize() runs move_matmul_waits_to_ldweights
    # + generate_event_semaphores, required on TRN2 (max 1 sync wait/inst).
    # reps>1 re-emits the whole body (idempotent) for slope-based HW timing.
    # bench_internal keeps the big inputs as Internal DRAM (no upload per
    # call; contents garbage — timing is data-independent).
    nc = bacc.Bacc()

    if bench_internal:
        XT8 = nc.dram_tensor("XT8", [128, KT, C8], FP8)
        XT16 = nc.dram_tensor("XT16", [128, KT, BT - C8], BF16)
        H0T = nc.dram_tensor("H0T", [H, Bp], F32R)
        WIH8 = nc.dram_tensor("WIH8", [128, KT, HP8], FP8)
        WIH16 = nc.dram_tensor("WIH16", [128, KT, H], BF16)
        WHH = nc.dram_tensor("WHH", [H, H], F32R)
        BIASH = nc.dram_tensor("BIASH", [H, 1], F32)
        WFC = nc.dram_tensor("WFC", [H, V], BF16)
        BFC = nc.dram_tensor("BFC", [1, V], BF16)
    else:
        XT8 = nc.declare_dram_parameter("XT8", [128, KT, C8], FP8, isOutput=False)
        XT16 = nc.declare_dram_parameter("XT16", [128, KT, BT - C8], BF16,
                                         isOutput=False)
        H0T = nc.declare_dram_parameter("H0T", [H, Bp], F32R, isOutput=False)
        WIH8 = nc.declare_dram_parameter("WIH8", [128, KT, HP8], FP8,
                                         isOutput=False)
        WIH16 = nc.declare_dram_parameter("WIH16", [128, KT, H], BF16,
                                          isOutput=False)
        WHH = nc.declare_dram_parameter("WHH", [H, H], F32R, isOutput=False)
        BIASH = nc.declare_dram_parameter("BIASH", [H, 1], F32, isOutput=False)
        WFC = nc.declare_dram_parameter("WFC", [H, V], BF16, isOutput=False)
        BFC = nc.declare_dram_parameter("BFC", [1, V], BF16, isOutput=False)
    IDEN = nc.declare_dram_parameter("IDEN", [128, 128], F32R, isOutput=False)
    ONES = nc.declare_dram_parameter("ONES", [1, Bp], BF16, isOutput=False)
    if yout_internal:
        # bench mode: full-size YOUT stays on-device; a tiny DOUT is the only
        # ExternalOutput so per-call host<->device traffic is negligible.
        # DOUT matches ONES' shape/dtype so bench.py can chain executions
        # (call i's DOUT feeds call i+1's ONES) inside one jit. EXTRA's
        # shape varies per build variant to defeat executable caching.
        YOUT = nc.dram_tensor("YOUT", [Bc, V], BF16)
        DOUT = nc.declare_dram_parameter("DOUT", [1, Bp], BF16, isOutput=True)
        EXTRA = nc.declare_dram_parameter(
            "EXTRA", [1, 1000 + reps], mybir.dt.uint8, isOutput=False)
    else:
        # bf16 output (upcast to f32 on the host): halves the store traffic
        # that phase 3 is otherwise bound on
        YOUT = nc.declare_dram_parameter("YOUT", [Bc, V], BF16, isOutput=True)
        DOUT = None

    with TileContext(nc) as tc:
      for _rep in range(reps):
        with tc.tile_pool(name="const", bufs=1) as cpool, \
             tc.tile_pool(name="hpool", bufs=2) as hpool:
            whh_a = cpool.tile([HA, H], F32R, tag="whh_a")
            whh_b = cpool.tile([HB, H], F32R, tag="whh_b")
            biash_a = cpool.tile([HA, 1], F32, tag="biash_a")
            biash_b = cpool.tile([HB, 1], F32, tag="biash_b")
            iden = cpool.tile([128, 128], F32R, tag="iden")
            xh_a = cpool.tile([HA, BT], F32R, tag="xh_a")
            xh_b = cpool.tile([HB, BT], F32R, tag="xh_b")

            nc.gpsimd.dma_start(out=whh_a, in_=WHH[0:HA, :])
            nc.gpsimd.dma_start(out=whh_b, in_=WHH[HA:H, :])
            nc.gpsimd.dma_start(out=biash_a, in_=BIASH[0:HA, :])
            nc.gpsimd.dma_start(out=biash_b, in_=BIASH[HA:H, :])
            nc.gpsimd.dma_start(out=iden, in_=IDEN[:, :])

            cur_a = hpool.tile([HA, Bp], F32R, tag="ha")
            cur_b = hpool.tile([HB, Bp], F32R, tag="hb")
            nc.gpsimd.dma_start(out=cur_a, in_=H0T[0:HA, :])
            nc.gpsimd.dma_start(out=cur_b, in_=H0T[HA:H, :])

            # FC weights (bf16): tiles allocated here, but the 4.3 MB of
            # DMAs are deferred into the phase-1 loop so they don't delay
            # the WIH load that gates the first matmul.  wfc_b carries the
            # bias b_fc as an extra contraction row (ones row in h8_b).
            fpool = tc.alloc_tile_pool(name="fc", bufs=1)
            wfc_a = fpool.tile([HA, V], BF16, tag="wfc_a")
            wfc_b = fpool.tile([HB + 1, V], BF16, tag="wfc_b")

            # ---- phase 1: xh.T = (W_ih.T).T @ X.T + bias, PSUM-accumulated
            #      (fp8-DoubleRow chunks for cols [0,C8), bf16 after)
            # ---- phase 2: 8 recurrence steps, interleaved into the phase-1
            #      chunk loop as soon as the xh columns they need are ready,
            #      so their PE/ACT latency hides under phase-1 matmuls.
            # chunk table: (is_fp8, global col0, width)
            chunks = []
            col0 = 0
            for w in CHUNKS8:
                chunks.append((True, col0, w))
                col0 += w
            for w in CHUNKS16:
                chunks.append((False, col0, w))
                col0 += w
            assert col0 == BT
            NCHT = len(chunks)
            rec_after = {}
            if phases in (2, 3):
                for t in range(T):
                    c_ready = next(c for c, (_, c0, w) in enumerate(chunks)
                                   if c0 + w >= Bp * (t + 1))
                    rec_after.setdefault(c_ready, []).append(t)

            def rec_step(t, ps2):
                nonlocal cur_a, cur_b
                new_a = hpool.tile([HA, Bp], F32R, tag="ha")
                new_b = hpool.tile([HB, Bp], F32R, tag="hb")
                for (c0, cn) in REC_CHUNKS:
                    p0 = ps2.tile([HA, 320], F32, tag="p0")
                    p1 = ps2.tile([HB, 320], F32, tag="p1")
                    col = t * Bp + c0
                    # h_new[0:128]
                    nc.tensor.matmul(
                        p0[:, 0:cn], whh_a[:, 0:HA], cur_a[:, c0:c0 + cn],
                        start=True, stop=False)
                    nc.tensor.matmul(
                        p0[:, 0:cn], whh_b[:, 0:HA], cur_b[:, c0:c0 + cn],
                        start=False, stop=False)
                    nc.tensor.matmul(
                        p0[:, 0:cn], iden, xh_a[:, col:col + cn],
                        start=False, stop=True)
                    # h_new[128:200]
                    nc.tensor.matmul(
                        p1[:, 0:cn], whh_a[:, HA:H], cur_a[:, c0:c0 + cn],
                        start=True, stop=False)
                    nc.tensor.matmul(
                        p1[:, 0:cn], whh_b[:, HA:H], cur_b[:, c0:c0 + cn],
                        start=False, stop=False)
                    nc.tensor.matmul(
                        p1[:, 0:cn], iden[0:HB, 0:HB], xh_b[:, col:col + cn],
                        start=False, stop=True)
                    nc.scalar.activation(
                        new_a[:, c0:c0 + cn], p0[:, 0:cn], AF.Tanh)
                    nc.scalar.activation(
                        new_b[:, c0:c0 + cn], p1[:, 0:cn], AF.Tanh)
                cur_a, cur_b = new_a, new_b

            with tc.tile_pool(name="wih", bufs=1) as wpool, \
                 tc.tile_pool(name="xslab", bufs=xbufs) as xpool, \
                 tc.tile_pool(name="ps1", bufs=2, space="PSUM") as ps1, \
                 tc.tile_pool(name="ps2", bufs=2, space="PSUM") as ps2:
                wih8_sb = wpool.tile([128, KT, HP8], FP8, tag="wih8")
                wih16_sb = wpool.tile([128, KT, H], BF16, tag="wih16")
                # split the WIH8 load so the first k-tiles' weights (which
                # gate the first matmul) land fast; WIH16 is deferred (not
                # needed until the bf16 chunks, ~2/3 into phase 1)
                for s in range(NSUB):
                    nc.gpsimd.dma_start(
                        out=wih8_sb[:, s * sub:(s + 1) * sub, :],
                        in_=WIH8[:, s * sub:(s + 1) * sub, :])

                for c in range(NCHT if phases != 4 else 0):
                    if c == 1:
                        # deferred loads: gpsimd ring is idle from here on
                        nc.gpsimd.dma_start(out=wih16_sb, in_=WIH16[:, :, :])
                        nc.gpsimd.dma_start(out=wfc_a, in_=WFC[0:HA, :])
                        nc.gpsimd.dma_start(
                            out=wfc_b[0:HB, :], in_=WFC[HA:H, :])
                        nc.gpsimd.dma_start(
                            out=wfc_b[HB:HB + 1, :], in_=BFC[:, :])
                    is8, col, w = chunks[c]
                    pa = ps1.tile([HA, 512], F32, tag="pa")
                    pb = ps1.tile([HB, 512], F32, tag="pb")
                    for s in range(KT // sub):
                        if is8:
                            xs = xpool.tile([128, sub, 512], FP8, tag="xs8")
                            nc.sync.dma_start(
                                out=xs[:, :, 0:w],
                                in_=XT8[:, s * sub:(s + 1) * sub,
                                        col:col + w],
                            )
                            for j2 in range(sub // 2):
                                p = (s * sub) // 2 + j2
                                st = (p == 0)
                                sp = (p == KT // 2 - 1)
                                nc.tensor.matmul(
                                    pa[:, 0:w],
                                    wih8_sb[:, 2 * p:2 * p + 2, 0:HA],
                                    xs[:, 2 * j2:2 * j2 + 2, 0:w],
                                    start=st, stop=sp, perf_mode=DR,
                                )
                                nc.tensor.matmul(
                                    pb[:, 0:w],
                                    wih8_sb[:, 2 * p:2 * p + 2, HA:H],
                                    xs[:, 2 * j2:2 * j2 + 2, 0:w],
                                    start=st, stop=sp, perf_mode=DR,
                                )
                        else:
                            xs = xpool.tile([128, sub, 512], BF16, tag="xs16")
                            nc.sync.dma_start(
                                out=xs[:, :, 0:w],
                                in_=XT16[:, s * sub:(s + 1) * sub,
                                         col - C8:col - C8 + w],
                            )
                            for j in range(sub):
                                k = s * sub + j
                                st = (k == 0)
                                sp = (k == KT - 1)
                                nc.tensor.matmul(
                                    pa[:, 0:w], wih16_sb[:, k, 0:HA],
                                    xs[:, j, 0:w],
                                    start=st, stop=sp,
                                )
                                nc.tensor.matmul(
                                    pb[:, 0:w], wih16_sb[:, k, HA:H],
                                    xs[:, j, 0:w],
                                    start=st, stop=sp,
                                )
                    nc.scalar.activation(
                        xh_a[:, col:col + w], pa[:, 0:w], AF.Identity,
                        bias=biash_a,
                    )
                    nc.scalar.activation(
                        xh_b[:, col:col + w], pb[:, 0:w], AF.Identity,
                        bias=biash_b,
                    )
                    for t in rec_after.get(c, []):
                        rec_step(t, ps2)

            # ---- phase 3: out = h_last @ W_fc.T + b_fc, natural layout
            if phases < 3:
                # still touch YOUT so outputs exist (gpsimd can cast f32r->f32)
                nc.gpsimd.dma_start(out=YOUT[0:HA, 0:Bp], in_=cur_a)
                fpool.release()
                continue
            with tc.tile_pool(name="outp", bufs=2) as opool, \
                 tc.tile_pool(name="ps3", bufs=4, space="PSUM") as ps3:
                # cast h_last to bf16 so FC stationaries use the fast
                # (FWL) weight-load path instead of ~1.1us fp32 self-loads.
                # h8_b gets a ones row so the wfc_b matmul also adds b_fc.
                h8_a = opool.tile([HA, Bp], BF16, tag="h8a", bufs=1)
                h8_b = opool.tile([HB + 1, Bp], BF16, tag="h8b", bufs=1)
                nc.vector.tensor_copy(h8_a, cur_a)
                nc.vector.tensor_copy(h8_b[0:HB, :], cur_b)
                nc.gpsimd.dma_start(out=h8_b[HB:HB + 1, :], in_=ONES[:, :])

                for bi, (b0, bn, bs) in enumerate(FC_BTILES):
                    yt = opool.tile([128, V], BF16, tag="yt")
                    for ci, (v0, vn) in enumerate(FC_CHUNKS):
                        pf = ps3.tile([128, 480], F32, tag="pf")
                        nc.tensor.matmul(
                            pf[0:bn, 0:vn], h8_a[:, b0:b0 + bn],
                            wfc_a[:, v0:v0 + vn], start=True,
                            stop=(fc_mode == 'mm1'))
                        if fc_mode != 'mm1':
                            nc.tensor.matmul(
                                pf[0:bn, 0:vn], h8_b[:, b0:b0 + bn],
                                wfc_b[:, v0:v0 + vn], start=False, stop=True)
                        if fc_mode in ('nostore', 'full'):
                            # alternate DVE / ACT so the PSUM drain keeps up
                            # with the matmuls (DVE alone is the pacer)
                            if ci % 2 == 0:
                                nc.vector.tensor_copy(
                                    yt[0:bn, v0:v0 + vn], pf[0:bn, 0:vn])
                            else:
                                nc.scalar.activation(
                                    yt[0:bn, v0:v0 + vn], pf[0:bn, 0:vn],
                                    AF.Identity)
                        else:
                            nc.vector.tensor_copy(
                                yt[0:bn, 0:8], pf[0:bn, 0:8])
                    if fc_mode == 'full':
                        # one bf16 store per b-tile, alternating HWDGE rings
                        eng = nc.sync if bi % 2 == 0 else nc.scalar
                        eng.dma_start(out=YOUT[b0:b0 + bs, :], in_=yt[0:bs, :])
                    else:
                        nc.scalar.dma_start(out=YOUT[b0:b0 + bs, 0:8],
                                            in_=yt[0:bs, 0:8])
            fpool.release()

      if yout_internal:
          with tc.tile_pool(name="dpool", bufs=1) as dpool:
              dt = dpool.tile([1, Bp], BF16, tag="dout")
              et = dpool.tile([1, 1000 + reps], mybir.dt.uint8, tag="extra")
              nc.sync.dma_start(out=et, in_=EXTRA[:, :])
              nc.sync.dma_start(out=dt, in_=YOUT[0:1, 0:Bp])
              nc.sync.dma_start(out=DOUT[:, :], in_=dt)

    nc.finalize()
    return nc


def _prep_host(X, h0, W_ih, W_hh, b_ih, b_hh, W_fc, b_fc):
    f = np.float32
    import ml_dtypes
    bf = ml_dtypes.bfloat16
    f8 = ml_dtypes.float8_e4m3
    X = np.asarray(X, f)
    # X.T slabs: XT*[core, p, k, t*Bp+b] = X[core*Bc+b, t, k*128+p]
    # (v zero-padded to VP, b zero-padded to Bp); timesteps < T8 in fp8,
    # the rest in bf16
    Xr = X.reshape(NCORE, Bc, T, V)

    def xt_region(t0, t1, dt):
        nt = t1 - t0
        srcp = np.zeros((NCORE, VP, nt, Bp), dt)
        srcp[:, :V, :, :Bc] = Xr[:, :, t0:t1].transpose(0, 3, 2, 1)
        srcp = srcp.reshape(NCORE, VP, nt * Bp)
        return np.ascontiguousarray(
            srcp.reshape(NCORE, KT, 128, nt * Bp).transpose(0, 2, 1, 3))

    XT8r = xt_region(0, T8, f8)
    XT16r = xt_region(T8, T, bf)

    wih_t8 = np.zeros((VP, HP8), f8)
    wih_t8[:V, :H] = np.asarray(W_ih, f).T                 # [v, h]
    WIH8r = np.ascontiguousarray(wih_t8.reshape(KT, 128, HP8).transpose(1, 0, 2))
    wih_t = np.zeros((VP, H), bf)
    wih_t[:V] = np.asarray(W_ih, f).T
    WIH16r = np.ascontiguousarray(wih_t.reshape(KT, 128, H).transpose(1, 0, 2))

    WHHt = np.ascontiguousarray(np.asarray(W_hh, f).T)     # [h_prev, h_new]
    BIASHv = (np.asarray(b_ih, f) + np.asarray(b_hh, f)).reshape(H, 1).copy()
    WFCt = np.ascontiguousarray(np.asarray(W_fc, ml_dtypes.bfloat16).T)  # [h, v]
    BFCv = np.asarray(b_fc, ml_dtypes.bfloat16).reshape(1, V).copy()
    H0T = np.zeros((NCORE, H, Bp), f)
    H0T[:, :, :Bc] = np.asarray(h0, f).reshape(NCORE, Bc, H).transpose(0, 2, 1)
    IDENv = np.eye(128, dtype=f)
    ONESv = np.ones((1, Bp), ml_dtypes.bfloat16)

    in_maps = []
    for i in range(NCORE):
        in_maps.append({
            "XT8": XT8r[i], "XT16": XT16r[i], "H0T": H0T[i],
            "WIH8": WIH8r, "WIH16": WIH16r, "WHH": WHHt,
            "BIASH": BIASHv, "WFC": WFCt, "BFC": BFCv, "IDEN": IDENv,
            "ONES": ONESv,
        })
    return in_maps


def kernel(X, h0, W_ih, W_hh, b_ih, b_hh, W_fc, b_fc):
    global LAST_RESULT
    in_maps = _prep_host(X, h0, W_ih, W_hh, b_ih, b_hh, W_fc, b_fc)
    if "nc" not in _CACHE:
        _CACHE["nc"] = _build()
    res = run_bass_kernel_spmd(_CACHE["nc"], in_maps, list(range(NCORE)))
    LAST_RESULT = res
    return np.concatenate(
        [res.results[i]["YOUT"].astype(np.float32) for i in range(NCORE)], axis=0)



# revision 39
# speedup vs baseline: 1.6923x; 1.6923x over previous
"""Trainium2 Bass kernel for batched tanh-RNN (B=5000, T=8, V=5264, H=200).

  xh   = X @ W_ih.T + b_ih + b_hh          # [B, T, H]  (bulk of FLOPs)
  h_t  = tanh(xh[:, t] + h_{t-1} @ W_hh.T) # 8 steps
  out  = h_T @ W_fc.T + b_fc               # [B, V]

Strategy: data-parallel over batch across 8 NeuronCores (625 rows each),
weights replicated.  On each core everything is computed in "transposed"
layout (hidden dim on partitions, batch on the free dim) so the recurrence
needs no on-chip transposes:

  phase 1: xh.T[h, t*625+b] accumulated in PSUM over 42 v-tiles of 128;
           stationary = W_ih.T tiles, moving = X.T slabs streamed from HBM.
           X is re-laid-out on the host to [128, 42, 5000] (v-major) so the
           DMA is 2KB-contiguous per partition.
  phase 2: h.T = tanh(W_hh.T.T @ h.T + xh_t.T); the xh_t term is added into
           the same PSUM accumulation group via an identity-stationary
           matmul, then one ACT Tanh PSUM->SBUF per tile.
  phase 3: out[b, v] = h.T-as-stationary @ W_fc.T tiles (+ b_fc via a
           ones-stationary K=1 matmul), natural-layout DMA store.

All matmul operands are float32r (FP22 single-pass, 1 cycle/row for moving
free dim >= 256) — ~1e-4 relative precision, full PE speed.
"""

import numpy as np

import concourse.bass as bass
import concourse.mybir as mybir
from concourse import bacc
from concourse.bass_utils import run_bass_kernel_spmd
from concourse.tile import TileContext

NCORE = 8
B, T, V, H = 5000, 8, 5264, 200
Bc = B // NCORE            # 625 batch rows per core
Bp = 640                   # padded to keep all fp32r APs 8-byte aligned
BT = Bp * T                # 5120 (t-major columns: col = t*Bp + b)
VP = 5376                  # V padded to 42*128
KT = VP // 128             # 42 contraction tiles
SUB = 14                   # v-tiles per streamed X slab
NSUB = KT // SUB           # 3 slabs per bt-chunk
HA, HB = 128, H - 128      # hidden split across partition tiles (128 + 72)

# Mixed-precision phase 1: timesteps 0..T8-1 stream X in fp8-e4m3 and run
# DoubleRow matmuls (2 fp8 K-rows/cycle, ~1.7x bf16); errors injected that
# early are attenuated to noise by the 2+ remaining rounds of tanh
# saturation (measured: rel_absmax 6.5e-3 vs 4.1e-3 all-bf16, gate 2e-2).
# The last T16 timesteps stay bf16.
# Chunk widths keep DMA descriptors >= 512 B (below that SDMA does RMW at
# half rate): fp8 1 B/col -> 512-col chunks; bf16 2 B/col -> 512/256 ok.
T8 = 6                     # timesteps in fp8
T16 = T - T8               # timesteps in bf16
C8 = Bp * T8               # fp8 column region [0, 3840)
HP8 = 208                  # W_ih fp8 h-stride padded to 16B multiple


def _chunk_widths(total):
    """Split into 512-wide chunks + a >=256 remainder (PSUM bank limit 512,
    fp32r/perf floors want >=256)."""
    out = [512] * (total // 512)
    r = total % 512
    if r >= 256:
        out.append(r)
    elif r > 0:
        out[-1:] = [(512 + r) // 2, (512 + r) - (512 + r) // 2]
    return out


CHUNKS8 = _chunk_widths(C8)           # 3840 -> 7x512 + 256
CHUNKS16 = _chunk_widths(BT - C8)     # 1280 -> 512, 512, 256
NCH8 = len(CHUNKS8)

F32 = mybir.dt.float32
F32R = mybir.dt.float32r
BF16 = mybir.dt.bfloat16
FP8 = mybir.dt.float8e4
AF = mybir.ActivationFunctionType
DR = mybir.MatmulPerfMode.DoubleRow

# recurrence b-chunks (even, >=256 so float32r runs 1 cycle/row)
REC_CHUNKS = [(0, 320), (320, 320)]
# FC output v-chunks (even offsets/widths, all >=256, <=512)
FC_CHUNKS = [(i * 480, 480) for i in range(10)] + [(4800, 464)]
# FC batch tiles over padded 640 (stationary free dim = 128; the last tile
# computes 15 pad rows that are simply not stored)
FC_BTILES = [(0, 128, 128), (128, 128, 128), (256, 128, 128),
             (384, 128, 128), (512, 128, 113)]

_CACHE = {}
LAST_RESULT = None  # BassKernelResults of the most recent run (for test.py)


def _build(reps=1, bench_internal=False, phases=3, sub=SUB, xbufs=3, fc_mode='full',
           yout_internal=False, no_dr=0):
    # Bacc (not raw Bass): its finalize() runs move_matmul_waits_to_ldweights
    # + generate_event_semaphores, required on TRN2 (max 1 sync wait/inst).
    # reps>1 re-emits the whole body (idempotent) for slope-based HW timing.
    # bench_internal keeps the big inputs as Internal DRAM (no upload per
    # call; contents garbage — timing is data-independent).
    nc = bacc.Bacc()

    if bench_internal:
        XT8 = nc.dram_tensor("XT8", [128, KT, C8], FP8)
        XT16 = nc.dram_tensor("XT16", [128, KT, BT - C8], BF16)
        H0T = nc.dram_tensor("H0T", [H, Bp], BF16)
        WIH8 = nc.dram_tensor("WIH8", [128, KT, HP8], FP8)
        WIH16 = nc.dram_tensor("WIH16", [128, KT, H], BF16)
        WHH = nc.dram_tensor("WHH", [H, H], BF16)
        BIASH = nc.dram_tensor("BIASH", [H, 1], F32)
        WFC = nc.dram_tensor("WFC", [H, V], BF16)
        BFC = nc.dram_tensor("BFC", [1, V], BF16)
    else:
        XT8 = nc.declare_dram_parameter("XT8", [128, KT, C8], FP8, isOutput=False)
        XT16 = nc.declare_dram_parameter("XT16", [128, KT, BT - C8], BF16,
                                         isOutput=False)
        H0T = nc.declare_dram_parameter("H0T", [H, Bp], BF16, isOutput=False)
        WIH8 = nc.declare_dram_parameter("WIH8", [128, KT, HP8], FP8,
                                         isOutput=False)
        WIH16 = nc.declare_dram_parameter("WIH16", [128, KT, H], BF16,
                                          isOutput=False)
        WHH = nc.declare_dram_parameter("WHH", [H, H], BF16, isOutput=False)
        BIASH = nc.declare_dram_parameter("BIASH", [H, 1], F32, isOutput=False)
        WFC = nc.declare_dram_parameter("WFC", [H, V], BF16, isOutput=False)
        BFC = nc.declare_dram_parameter("BFC", [1, V], BF16, isOutput=False)
    IDEN = nc.declare_dram_parameter("IDEN", [128, 128], BF16, isOutput=False)
    ONES = nc.declare_dram_parameter("ONES", [1, Bp], BF16, isOutput=False)
    if yout_internal:
        # bench mode: full-size YOUT stays on-device; a tiny DOUT is the only
        # ExternalOutput so per-call host<->device traffic is negligible.
        # DOUT matches ONES' shape/dtype so bench.py can chain executions
        # (call i's DOUT feeds call i+1's ONES) inside one jit. EXTRA's
        # shape varies per build variant to defeat executable caching.
        YOUT = nc.dram_tensor("YOUT", [Bc, V], BF16)
        DOUT = nc.declare_dram_parameter("DOUT", [1, Bp], BF16, isOutput=True)
        EXTRA = nc.declare_dram_parameter(
            "EXTRA", [1, 1000 + reps], mybir.dt.uint8, isOutput=False)
    else:
        # bf16 output (upcast to f32 on the host): halves the store traffic
        # that phase 3 is otherwise bound on
        YOUT = nc.declare_dram_parameter("YOUT", [Bc, V], BF16, isOutput=True)
        DOUT = None

    with TileContext(nc) as tc:
      for _rep in range(reps):
        with tc.tile_pool(name="const", bufs=1) as cpool, \
             tc.tile_pool(name="hpool", bufs=2) as hpool:
            whh_a = cpool.tile([HA, H], BF16, tag="whh_a")
            whh_b = cpool.tile([HB, H], BF16, tag="whh_b")
            biash_a = cpool.tile([HA, 1], F32, tag="biash_a")
            biash_b = cpool.tile([HB, 1], F32, tag="biash_b")
            iden = cpool.tile([128, 128], BF16, tag="iden")
            xh_a = cpool.tile([HA, BT], BF16, tag="xh_a")
            xh_b = cpool.tile([HB, BT], BF16, tag="xh_b")

            cur_a = hpool.tile([HA, Bp], BF16, tag="ha")
            cur_b = hpool.tile([HB, Bp], BF16, tag="hb")

            # FC weights (bf16): tiles allocated here, but the 4.3 MB of
            # DMAs are deferred into the phase-1 loop so they don't delay
            # the WIH load that gates the first matmul.  wfc_b carries the
            # bias b_fc as an extra contraction row (ones row in h8_b).
            fpool = tc.alloc_tile_pool(name="fc", bufs=1)
            wfc_a = fpool.tile([HA, V], BF16, tag="wfc_a")
            wfc_b = fpool.tile([HB + 1, V], BF16, tag="wfc_b")

            # ---- phase 1: xh.T = (W_ih.T).T @ X.T + bias, PSUM-accumulated
            #      (fp8-DoubleRow chunks for cols [0,C8), bf16 after)
            # ---- phase 2: 8 recurrence steps, interleaved into the phase-1
            #      chunk loop as soon as the xh columns they need are ready,
            #      so their PE/ACT latency hides under phase-1 matmuls.
            # chunk table: (is_fp8, global col0, width)
            chunks = []
            col0 = 0
            for w in CHUNKS8:
                chunks.append((True, col0, w))
                col0 += w
            for w in CHUNKS16:
                chunks.append((False, col0, w))
                col0 += w
            assert col0 == BT
            NCHT = len(chunks)
            rec_after = {}
            if phases in (2, 3):
                for t in range(T):
                    c_ready = next(c for c, (_, c0, w) in enumerate(chunks)
                                   if c0 + w >= Bp * (t + 1))
                    rec_after.setdefault(c_ready, []).append(t)

            def rec_step(t, ps2):
                nonlocal cur_a, cur_b
                new_a = hpool.tile([HA, Bp], BF16, tag="ha")
                new_b = hpool.tile([HB, Bp], BF16, tag="hb")
                for (c0, cn) in REC_CHUNKS:
                    p0 = ps2.tile([HA, 320], F32, tag="p0")
                    p1 = ps2.tile([HB, 320], F32, tag="p1")
                    col = t * Bp + c0
                    # h_new[0:128]
                    nc.tensor.matmul(
                        p0[:, 0:cn], whh_a[:, 0:HA], cur_a[:, c0:c0 + cn],
                        start=True, stop=False)
                    nc.tensor.matmul(
                        p0[:, 0:cn], whh_b[:, 0:HA], cur_b[:, c0:c0 + cn],
                        start=False, stop=False)
                    nc.tensor.matmul(
                        p0[:, 0:cn], iden, xh_a[:, col:col + cn],
                        start=False, stop=True)
                    # h_new[128:200]
                    nc.tensor.matmul(
                        p1[:, 0:cn], whh_a[:, HA:H], cur_a[:, c0:c0 + cn],
                        start=True, stop=False)
                    nc.tensor.matmul(
                        p1[:, 0:cn], whh_b[:, HA:H], cur_b[:, c0:c0 + cn],
                        start=False, stop=False)
                    nc.tensor.matmul(
                        p1[:, 0:cn], iden[0:HB, 0:HB], xh_b[:, col:col + cn],
                        start=False, stop=True)
                    nc.scalar.activation(
                        new_a[:, c0:c0 + cn], p0[:, 0:cn], AF.Tanh)
                    nc.scalar.activation(
                        new_b[:, c0:c0 + cn], p1[:, 0:cn], AF.Tanh)
                cur_a, cur_b = new_a, new_b

            with tc.tile_pool(name="wih", bufs=1) as wpool, \
                 tc.tile_pool(name="xslab", bufs=xbufs) as xpool, \
                 tc.tile_pool(name="ps1", bufs=2, space="PSUM") as ps1, \
                 tc.tile_pool(name="ps2", bufs=2, space="PSUM") as ps2:
                wih8_sb = wpool.tile([128, KT, HP8], FP8, tag="wih8")
                wih16_sb = wpool.tile([128, KT, H], BF16, tag="wih16")
                # WIH8 goes FIRST on the gpsimd ring (its first chunk gates
                # the very first matmul); small constants follow; WIH16 is
                # deferred (not needed until the bf16 chunks ~2/3 in)
                for s in range(NSUB):
                    nc.gpsimd.dma_start(
                        out=wih8_sb[:, s * sub:(s + 1) * sub, :],
                        in_=WIH8[:, s * sub:(s + 1) * sub, :])
                # small constants ride the scalar HWDGE ring (0.6us fixed vs
                # SWDGE's ~2us, and it parallelizes with the wih8 loads; the
                # ring is otherwise empty until the phase-3 stores)
                nc.scalar.dma_start(out=whh_a, in_=WHH[0:HA, :])
                nc.scalar.dma_start(out=whh_b, in_=WHH[HA:H, :])
                nc.scalar.dma_start(out=biash_a, in_=BIASH[0:HA, :])
                nc.scalar.dma_start(out=biash_b, in_=BIASH[HA:H, :])
                nc.scalar.dma_start(out=iden, in_=IDEN[:, :])
                nc.scalar.dma_start(out=cur_a, in_=H0T[0:HA, :])
                nc.scalar.dma_start(out=cur_b, in_=H0T[HA:H, :])

                if phases == 4:
                    # bench floor: still load the deferred weights so their
                    # tiles aren't released unwritten
                    nc.gpsimd.dma_start(out=wih16_sb, in_=WIH16[:, :, :])
                    nc.gpsimd.dma_start(out=wfc_a, in_=WFC[0:HA, :])
                    nc.gpsimd.dma_start(out=wfc_b[0:HB, :], in_=WFC[HA:H, :])
                    nc.gpsimd.dma_start(out=wfc_b[HB:HB + 1, :], in_=BFC[:, :])
                for c in range(NCHT if phases != 4 else 0):
                    if c == 1:
                        # deferred loads: gpsimd ring is idle from here on
                        nc.gpsimd.dma_start(out=wih16_sb, in_=WIH16[:, :, :])
                        nc.gpsimd.dma_start(out=wfc_a, in_=WFC[0:HA, :])
                        nc.gpsimd.dma_start(
                            out=wfc_b[0:HB, :], in_=WFC[HA:H, :])
                        nc.gpsimd.dma_start(
                            out=wfc_b[HB:HB + 1, :], in_=BFC[:, :])
                    is8, col, w = chunks[c]
                    pa = ps1.tile([HA, 512], F32, tag="pa")
                    pb = ps1.tile([HB, 512], F32, tag="pb")
                    for s in range(KT // sub):
                        if is8:
                            xs = xpool.tile([128, sub, 512], FP8, tag="xs8")
                            nc.sync.dma_start(
                                out=xs[:, :, 0:w],
                                in_=XT8[:, s * sub:(s + 1) * sub,
                                        col:col + w],
                            )
                            if no_dr:
                                # A/B: fp8 without DoubleRow (1 cyc/row)
                                for j in range(sub):
                                    k = s * sub + j
                                    st = (k == 0)
                                    sp = (k == KT - 1)
                                    nc.tensor.matmul(
                                        pa[:, 0:w], wih8_sb[:, k, 0:HA],
                                        xs[:, j, 0:w], start=st, stop=sp)
                                    nc.tensor.matmul(
                                        pb[:, 0:w], wih8_sb[:, k, HA:H],
                                        xs[:, j, 0:w], start=st, stop=sp)
                                continue
                            for j2 in range(sub // 2):
                                p = (s * sub) // 2 + j2
                                st = (p == 0)
                                sp = (p == KT // 2 - 1)
                                nc.tensor.matmul(
                                    pa[:, 0:w],
                                    wih8_sb[:, 2 * p:2 * p + 2, 0:HA],
                                    xs[:, 2 * j2:2 * j2 + 2, 0:w],
                                    start=st, stop=sp, perf_mode=DR,
                                )
                                nc.tensor.matmul(
                                    pb[:, 0:w],
                                    wih8_sb[:, 2 * p:2 * p + 2, HA:H],
                                    xs[:, 2 * j2:2 * j2 + 2, 0:w],
                                    start=st, stop=sp, perf_mode=DR,
                                )
                        else:
                            xs = xpool.tile([128, sub, 512], BF16, tag="xs16")
                            nc.sync.dma_start(
                                out=xs[:, :, 0:w],
                                in_=XT16[:, s * sub:(s + 1) * sub,
                                         col - C8:col - C8 + w],
                            )
                            for j in range(sub):
                                k = s * sub + j
                                st = (k == 0)
                                sp = (k == KT - 1)
                                nc.tensor.matmul(
                                    pa[:, 0:w], wih16_sb[:, k, 0:HA],
                                    xs[:, j, 0:w],
                                    start=st, stop=sp,
                                )
                                nc.tensor.matmul(
                                    pb[:, 0:w], wih16_sb[:, k, HA:H],
                                    xs[:, j, 0:w],
                                    start=st, stop=sp,
                                )
                    # epilogues split ACT/DVE so neither engine's queue
                    # delays the interleaved recurrence steps
                    nc.scalar.activation(
                        xh_a[:, col:col + w], pa[:, 0:w], AF.Identity,
                        bias=biash_a,
                    )
                    nc.vector.tensor_scalar_add(
                        xh_b[:, col:col + w], pb[:, 0:w], biash_b)
                    for t in rec_after.get(c, []):
                        rec_step(t, ps2)

            # ---- phase 3: out = h_last @ W_fc.T + b_fc, natural layout
            if phases < 3:
                # still touch YOUT so outputs exist (gpsimd can cast f32r->f32)
                nc.gpsimd.dma_start(out=YOUT[0:HA, 0:Bp], in_=cur_a)
                fpool.release()
                continue
            with tc.tile_pool(name="outp", bufs=2) as opool, \
                 tc.tile_pool(name="ps3", bufs=4, space="PSUM") as ps3:
                # cast h_last to bf16 so FC stationaries use the fast
                # (FWL) weight-load path instead of ~1.1us fp32 self-loads.
                # h8_b gets a ones row so the wfc_b matmul also adds b_fc.
                h8_a = cur_a  # already bf16
                h8_b = opool.tile([HB + 1, Bp], BF16, tag="h8b", bufs=1)
                nc.vector.tensor_copy(h8_b[0:HB, :], cur_b)
                nc.gpsimd.dma_start(out=h8_b[HB:HB + 1, :], in_=ONES[:, :])

                for bi, (b0, bn, bs) in enumerate(FC_BTILES):
                    yt = opool.tile([128, V], BF16, tag="yt", bufs=3)
                    # process v-chunks in pairs sharing each stationary load
                    # (alternating h8_a/h8_b per MM would force an LDWEIGHTS
                    # into every matmul slot)
                    pairs = [FC_CHUNKS[i:i + 2]
                             for i in range(0, len(FC_CHUNKS), 2)]
                    for pi, pair in enumerate(pairs):
                        pfs = []
                        for q in range(len(pair)):
                            pfq = ps3.tile([128, 480], F32, tag=f"pf{q}",
                                           name=f"pf{q}")
                            pfs.append(pfq)
                        for pf, (v0, vn) in zip(pfs, pair):
                            nc.tensor.matmul(
                                pf[0:bn, 0:vn], h8_a[:, b0:b0 + bn],
                                wfc_a[:, v0:v0 + vn], start=True,
                                stop=(fc_mode == 'mm1'))
                        if fc_mode != 'mm1':
                            for pf, (v0, vn) in zip(pfs, pair):
                                nc.tensor.matmul(
                                    pf[0:bn, 0:vn], h8_b[:, b0:b0 + bn],
                                    wfc_b[:, v0:v0 + vn], start=False,
                                    stop=True)
                        for q, (pf, (v0, vn)) in enumerate(zip(pfs, pair)):
                            if fc_mode in ('nostore', 'full'):
                                # alternate DVE / ACT so the PSUM drain
                                # keeps up with the matmuls
                                if q % 2 == 0:
                                    nc.vector.tensor_copy(
                                        yt[0:bn, v0:v0 + vn], pf[0:bn, 0:vn])
                                else:
                                    nc.scalar.activation(
                                        yt[0:bn, v0:v0 + vn], pf[0:bn, 0:vn],
                                        AF.Identity)
                            else:
                                nc.vector.tensor_copy(
                                    yt[0:bn, 0:8], pf[0:bn, 0:8])
                    if fc_mode == 'full':
                        # stores go on the scalar HWDGE ring ONLY: the sync
                        # ring carries the X-slab loads (FIFO per ring, so a
                        # store there would stall the next rep's phase 1).
                        # Two halves per b-tile: the first issues as soon as
                        # chunks 0..5 are drained, shortening the tail.
                        vh = FC_CHUNKS[5][0] + FC_CHUNKS[5][1]
                        nc.scalar.dma_start(out=YOUT[b0:b0 + bs, 0:vh],
                                            in_=yt[0:bs, 0:vh])
                        nc.scalar.dma_start(out=YOUT[b0:b0 + bs, vh:V],
                                            in_=yt[0:bs, vh:V])
                    else:
                        nc.scalar.dma_start(out=YOUT[b0:b0 + bs, 0:8],
                                            in_=yt[0:bs, 0:8])
            fpool.release()

      if yout_internal:
          with tc.tile_pool(name="dpool", bufs=1) as dpool:
              dt = dpool.tile([1, Bp], BF16, tag="dout")
              et = dpool.tile([1, 1000 + reps], mybir.dt.uint8, tag="extra")
              nc.sync.dma_start(out=et, in_=EXTRA[:, :])
              nc.sync.dma_start(out=dt, in_=YOUT[0:1, 0:Bp])
              nc.sync.dma_start(out=DOUT[:, :], in_=dt)

    nc.finalize()
    return nc


def _prep_host(X, h0, W_ih, W_hh, b_ih, b_hh, W_fc, b_fc):
    f = np.float32
    import ml_dtypes
    bf = ml_dtypes.bfloat16
    f8 = ml_dtypes.float8_e4m3
    X = np.asarray(X, f)
    # X.T slabs: XT*[core, p, k, t*Bp+b] = X[core*Bc+b, t, k*128+p]
    # (v zero-padded to VP, b zero-padded to Bp); timesteps < T8 in fp8,
    # the rest in bf16
    Xr = X.reshape(NCORE, Bc, T, V)

    def xt_region(t0, t1, dt):
        nt = t1 - t0
        srcp = np.zeros((NCORE, VP, nt, Bp), dt)
        srcp[:, :V, :, :Bc] = Xr[:, :, t0:t1].transpose(0, 3, 2, 1)
        srcp = srcp.reshape(NCORE, VP, nt * Bp)
        return np.ascontiguousarray(
            srcp.reshape(NCORE, KT, 128, nt * Bp).transpose(0, 2, 1, 3))

    XT8r = xt_region(0, T8, f8)
    XT16r = xt_region(T8, T, bf)

    wih_t8 = np.zeros((VP, HP8), f8)
    wih_t8[:V, :H] = np.asarray(W_ih, f).T                 # [v, h]
    WIH8r = np.ascontiguousarray(wih_t8.reshape(KT, 128, HP8).transpose(1, 0, 2))
    wih_t = np.zeros((VP, H), bf)
    wih_t[:V] = np.asarray(W_ih, f).T
    WIH16r = np.ascontiguousarray(wih_t.reshape(KT, 128, H).transpose(1, 0, 2))

    WHHt = np.ascontiguousarray(np.asarray(W_hh, bf).T)    # [h_prev, h_new]
    BIASHv = (np.asarray(b_ih, f) + np.asarray(b_hh, f)).reshape(H, 1).copy()
    WFCt = np.ascontiguousarray(np.asarray(W_fc, ml_dtypes.bfloat16).T)  # [h, v]
    BFCv = np.asarray(b_fc, ml_dtypes.bfloat16).reshape(1, V).copy()
    H0T = np.zeros((NCORE, H, Bp), bf)
    H0T[:, :, :Bc] = np.asarray(h0, f).reshape(NCORE, Bc, H).transpose(0, 2, 1).astype(bf)
    IDENv = np.eye(128, dtype=bf)
    ONESv = np.ones((1, Bp), ml_dtypes.bfloat16)

    in_maps = []
    for i in range(NCORE):
        in_maps.append({
            "XT8": XT8r[i], "XT16": XT16r[i], "H0T": H0T[i],
            "WIH8": WIH8r, "WIH16": WIH16r, "WHH": WHHt,
            "BIASH": BIASHv, "WFC": WFCt, "BFC": BFCv, "IDEN": IDENv,
            "ONES": ONESv,
        })
    return in_maps


def kernel(X, h0, W_ih, W_hh, b_ih, b_hh, W_fc, b_fc):
    global LAST_RESULT
    in_maps = _prep_host(X, h0, W_ih, W_hh, b_ih, b_hh, W_fc, b_fc)
    if "nc" not in _CACHE:
        _CACHE["nc"] = _build()
    res = run_bass_kernel_spmd(_CACHE["nc"], in_maps, list(range(NCORE)))
    LAST_RESULT = res
    return np.concatenate(
        [res.results[i]["YOUT"].astype(np.float32) for i in range(NCORE)], axis=0)

